# revision 1
# baseline (speedup 1.0000x reference)
"""Trainium2 Bass kernel for nn_Attention_53188874993896 (sparse_attention).

Math notes (derived from the reference):
  - pos_scores[b,h,s,t] = r[b,h,s] - r[b,h,t] + head_b[h] with
    r = p @ head_w[h].  The s-dependent part is constant along the softmax
    axis t, so pos_attn is independent of s: pos_attn[b,h,s,t] = w[b,h,t]
    where w = softmax_t(-r).  Its whole contribution to the output is a
    per-batch vector C[b,d] (rank-1 collapse).
  - blend a = (1-g)*attn + g*pos_attn already has rows summing to 1, so the
    reference's renormalization is an identity up to float rounding.
  - softmax without max-subtraction is safe: |scores| <~ 4.

Sharding: data-parallel over batch B=64 across 8 cores (8 batches/core).
"""

import sys

sys.path.insert(0, "/opt/trn_rl_repo")

import numpy as np
import ml_dtypes

B, S, D, H, PD = 64, 256, 256, 8, 8
HD = D // H  # 32
P8 = D // 8  # 32
NCORES = 8
NB = B // NCORES  # batches per core
SCALE = 1.0 / np.sqrt(np.float32(HD))

bf16 = ml_dtypes.bfloat16

_CACHE = {}


def _build(nb, stage=99):
    import os
    import concourse.bass as bass  # noqa: F401
    import concourse.bacc as bacc
    import concourse.mybir as mybir
    from concourse.tile import TileContext

    fp32 = mybir.dt.float32
    bf = mybir.dt.bfloat16
    Exp = mybir.ActivationFunctionType.Exp

    nc = bacc.Bacc("TRN2", target_bir_lowering=False, debug=False)

    # ---- DRAM I/O ----
    x_d = nc.dram_tensor("x", [nb, S, D], fp32, kind="ExternalInput")
    pos_d = nc.dram_tensor("pos", [nb, S, PD], fp32, kind="ExternalInput")
    wq_d = nc.dram_tensor("wqT", [D, D], bf, kind="ExternalInput")  # [in,out]
    wk_d = nc.dram_tensor("wkT", [D, D], bf, kind="ExternalInput")
    vt_d = nc.dram_tensor("vT", [D, D], bf, kind="ExternalInput")  # (1-g) folded
    owt_d = nc.dram_tensor("owT", [D, D], bf, kind="ExternalInput")  # plain out_w.T
    owtg_d = nc.dram_tensor("owTg", [D, D], bf, kind="ExternalInput")  # * g/(1-g)
    w1_d = nc.dram_tensor("w1T", [PD, PD], bf, kind="ExternalInput")
    b1_d = nc.dram_tensor("b1c", [PD, 1], fp32, kind="ExternalInput")
    w2_d = nc.dram_tensor("w2T", [PD, P8], bf, kind="ExternalInput")
    hw_d = nc.dram_tensor("hwNeg", [P8, H], bf, kind="ExternalInput")
    hb2_d = nc.dram_tensor("hb2c", [H, 1], fp32, kind="ExternalInput")
    outb_d = nc.dram_tensor("outbRow", [1, D], bf, kind="ExternalInput")
    id_d = nc.dram_tensor("ident", [128, 128], bf, kind="ExternalInput")
    id32_d = nc.dram_tensor("ident32", [128, 128], fp32, kind="ExternalInput")
    ones_d = nc.dram_tensor("onesLhs", [128, 128], bf, kind="ExternalInput")
    out_d = nc.dram_tensor("out", [nb, S, D], fp32, kind="ExternalOutput")

    with TileContext(nc) as tc:
        with (
            tc.tile_pool(name="wsb", bufs=1) as wsb,
            tc.tile_pool(name="xin", bufs=2) as xin,
            tc.tile_pool(name="xtp", bufs=2) as xtp,
            tc.tile_pool(name="qkv", bufs=2) as qkv,
            tc.tile_pool(name="esb", bufs=4) as esb,
            tc.tile_pool(name="bld", bufs=2) as bld,
            tc.tile_pool(name="small", bufs=2) as small,
            tc.tile_pool(name="osb", bufs=2) as osb,
            tc.tile_pool(name="ps", bufs=1, space="PSUM") as ps,
        ):
            # ---- resident weights (SBUF, loaded once) ----
            wq_sb = wsb.tile([128, 2, D], bf, tag="wq")
            wk_sb = wsb.tile([128, 2, D], bf, tag="wk")
            vt_sb = wsb.tile([128, 2, D], bf, tag="vt")
            owt_sb = wsb.tile([128, 2, D], bf, tag="owt")
            owtg_sb = wsb.tile([128, 2, D], bf, tag="owtg")
            w1_sb = wsb.tile([PD, PD], bf, tag="w1")
            b1_sb = wsb.tile([PD, 1], fp32, tag="b1")
            w2_sb = wsb.tile([PD, P8], bf, tag="w2")
            hw_sb = wsb.tile([P8, H], bf, tag="hw")
            hb2_sb = wsb.tile([H, 1], fp32, tag="hb2")
            outb_sb = wsb.tile([1, D], bf, tag="outb")
            id_sb = wsb.tile([128, 128], bf, tag="id")
            id32_sb = wsb.tile([128, 128], fp32, tag="id32")
            ones_sb = wsb.tile([128, 128], bf, tag="ones")
            for t, d in (
                (wq_sb, wq_d), (wk_sb, wk_d), (vt_sb, vt_d),
                (owt_sb, owt_d), (owtg_sb, owtg_d),
            ):
                nc.sync.dma_start(out=t, in_=d.rearrange("(c p) o -> p c o", p=128))
            nc.sync.dma_start(out=w1_sb, in_=w1_d[:, :])
            nc.sync.dma_start(out=b1_sb, in_=b1_d[:, :])
            nc.sync.dma_start(out=w2_sb, in_=w2_d[:, :])
            nc.sync.dma_start(out=hw_sb, in_=hw_d[:, :])
            nc.sync.dma_start(out=hb2_sb, in_=hb2_d[:, :])
            nc.sync.dma_start(out=outb_sb, in_=outb_d[:, :])
            nc.sync.dma_start(out=id_sb, in_=id_d[:, :])
            nc.sync.dma_start(out=id32_sb, in_=id32_d[:, :])
            nc.sync.dma_start(out=ones_sb, in_=ones_d[:, :])

            # head h -> score-slot (spreads concurrent row-group MMs over banks)
            slot = [2 * (h % 4) + h // 4 for h in range(H)]

            # ---- pos branch: batched MLP over all nb batches ----
            if stage >= 2:
                pos_all = wsb.tile([128, nb, 2, PD], bf, tag="posall")
                nc.gpsimd.dma_start(
                    out=pos_all,
                    in_=pos_d.rearrange("b (c p) i -> p b c i", p=128))
                posT_sb = wsb.tile([PD, nb, S], bf, tag="posT")  # [i, b, s]
                h1_sb = wsb.tile([PD, nb, S], bf, tag="h1")
                pT_sb = wsb.tile([P8, nb, S], bf, tag="pT")
                w_all = wsb.tile([H, nb, S], bf, tag="wall")  # exp(-r), unnorm
                wcol_sb = wsb.tile([128, nb, 2, H], bf, tag="wcol")
                for b0 in range(0, nb, 2):
                    w = min(2, nb - b0)
                    pt_ps = ps.tile([PD, 4, 128], bf, tag="final")
                    for k in range(w):
                        for c in range(2):
                            nc.tensor.transpose(
                                pt_ps[:, 2 * k + c, :],
                                pos_all[:, b0 + k, c, :], id_sb)
                    nc.vector.tensor_copy(
                        posT_sb[:, b0:b0 + w, :].rearrange("i b s -> i (b s)"),
                        pt_ps[:, 0:2 * w, :].rearrange("i k t -> i (k t)"))
                    h1_ps = ps.tile([PD, 512], fp32, tag="final")
                    nc.tensor.matmul(
                        h1_ps[:, 0:256 * w], lhsT=w1_sb,
                        rhs=posT_sb[:, b0:b0 + w, :].rearrange("i b s -> i (b s)"),
                        start=True, stop=True)
                    nc.vector.tensor_scalar(
                        out=h1_sb[:, b0:b0 + w, :].rearrange("i b s -> i (b s)"),
                        in0=h1_ps[:, 0:256 * w], scalar1=b1_sb, scalar2=0.0,
                        op0=mybir.AluOpType.add, op1=mybir.AluOpType.max)
                    p_ps = ps.tile([P8, 512], fp32, tag="final")
                    nc.tensor.matmul(
                        p_ps[:, 0:256 * w], lhsT=w2_sb,
                        rhs=h1_sb[:, b0:b0 + w, :].rearrange("i b s -> i (b s)"),
                        start=True, stop=True)
                    nc.vector.tensor_copy(
                        pT_sb[:, b0:b0 + w, :].rearrange("c b s -> c (b s)"),
                        p_ps[:, 0:256 * w])
                    r_ps = ps.tile([H, 512], fp32, tag="final")
                    nc.tensor.matmul(
                        r_ps[:, 0:256 * w], lhsT=hw_sb,
                        rhs=pT_sb[:, b0:b0 + w, :].rearrange("c b s -> c (b s)"),
                        start=True, stop=True)
                    nc.scalar.activation(
                        w_all[:, b0:b0 + w, :].rearrange("h b s -> h (b s)"),
                        r_ps[:, 0:256 * w], Exp, bias=hb2_sb)
                    wt_ps = ps.tile([128, 4, H], bf, tag="final")
                    for k in range(w):
                        for c in range(2):
                            nc.tensor.transpose(
                                wt_ps[:, 2 * k + c, :],
                                w_all[:, b0 + k, 128 * c:128 * (c + 1)],
                                id_sb[0:H, 0:H])
                    nc.vector.tensor_copy(
                        wcol_sb[:, b0:b0 + w, :, :].rearrange(
                            "p b c h -> p (b c h)"),
                        wt_ps[:, 0:2 * w, :].rearrange("p k h -> p (k h)"))

            for b in range(nb):
                # ---- load + cast x ----
                x_bf = xin.tile([128, 2, D], bf, tag="x")
                with tc.high_priority():
                    nc.gpsimd.dma_start(
                        out=x_bf, in_=x_d[b].rearrange("(c p) d -> p c d", p=128))

                # ---- transpose x on PE (DMA xbar-transpose serializes the
                #      whole DMA stream on mode switches; PE is ~10x better
                #      here) ----
                xt_bf = xtp.tile([128, 2, S], bf, tag="xt")  # [i%128, i//128, s]
                xt_ps = ps.tile([128, 4, 128], bf, tag="misc")
                for cs in range(2):
                    for cd in range(2):
                        nc.tensor.transpose(
                            xt_ps[:, 2 * cs + cd, :],
                            x_bf[:, cs, 128 * cd:128 * (cd + 1)], id_sb)
                nc.vector.tensor_copy(
                    xt_bf.rearrange("p c (a t) -> p a c t", a=2), xt_ps)

                def dump(t):  # ablation: keep `out` written so nothing DCEs
                    nc.gpsimd.dma_start(
                        out=out_d[b].rearrange("(c p) d -> p c d", p=128),
                        in_=t)

                if stage < 1:
                    dump(xt_bf)
                    continue
                # ---- q/k/v projections ----
                q_ps = ps.tile([128, 2, S], fp32, tag="proj")
                for cm in range(2):
                    for ci in range(2):
                        nc.tensor.matmul(
                            q_ps[:, cm, :],
                            lhsT=wq_sb[:, ci, 128 * cm:128 * (cm + 1)],
                            rhs=xt_bf[:, ci, :],
                            start=(ci == 0), stop=(ci == 1))
                qT_sb = qkv.tile([128, 2, S], bf, tag="q")
                nc.vector.tensor_copy(qT_sb, q_ps)

                k_ps = ps.tile([128, 2, S], fp32, tag="proj")
                for cm in range(2):
                    for ci in range(2):
                        nc.tensor.matmul(
                            k_ps[:, cm, :],
                            lhsT=wk_sb[:, ci, 128 * cm:128 * (cm + 1)],
                            rhs=xt_bf[:, ci, :],
                            start=(ci == 0), stop=(ci == 1))
                kT_sb = qkv.tile([128, 2, S], bf, tag="k")
                nc.vector.tensor_copy(kT_sb, k_ps)

                v_ps = ps.tile([128, 2, D], fp32, tag="proj")
                for ct in range(2):
                    for ci in range(2):
                        nc.tensor.matmul(
                            v_ps[:, ct, :],
                            lhsT=xt_bf[:, ci, 128 * ct:128 * (ct + 1)],
                            rhs=vt_sb[:, ci, :],
                            start=(ci == 0), stop=(ci == 1))
                v_sb = qkv.tile([128, 2, D], bf, tag="v")  # [t%128, t//128, j]
                nc.vector.tensor_copy(v_sb, v_ps)

                if stage < 3:
                    dump(v_sb)
                    continue

                # ---- scores + exp (per t-chunk) ----
                exp_c = []
                for ct in range(2):
                    sc_ps = ps.tile([128, 2048], fp32, tag="scores")
                    for h in range(H):
                        rg = h % 4
                        nc.tensor.matmul(
                            sc_ps[:, 256 * slot[h]:256 * (slot[h] + 1)],
                            lhsT=kT_sb[32 * rg:32 * (rg + 1), h // 4,
                                       128 * ct:128 * (ct + 1)],
                            rhs=qT_sb[32 * rg:32 * (rg + 1), h // 4, :],
                            start=True, stop=True,
                            tile_position=(32 * rg, 0))
                    e_sb = esb.tile([128, H, S + 1], bf, tag="exp")
                    nc.scalar.activation(
                        e_sb[:, :, 0:S], sc_ps.rearrange("p (h s) -> p h s", h=H),
                        Exp, scale=float(SCALE))
                    # w column (precomputed in the pos phase)
                    nc.gpsimd.tensor_copy(
                        e_sb[:, :, S:S + 1], wcol_sb[:, b, ct, :])
                    exp_c.append(e_sb)

                if stage < 4:
                    dump(exp_c[0][:, 0:2, 0:256])
                    continue
                # ---- denominators (incl wsum in col 256) then ctx ----
                den_ps = []
                for q in range(2):
                    d_ps = ps.tile([128, S + 1], fp32, tag="ctx")
                    for cg in range(4):
                        h = 4 * q + cg
                        for ct in range(2):
                            nc.tensor.matmul(
                                d_ps[32 * cg:32 * (cg + 1), :],
                                lhsT=ones_sb[:, 0:32],
                                rhs=exp_c[ct][:, slot[h], :],
                                start=(ct == 0), stop=(ct == 1),
                                tile_position=(0, 32 * cg))
                    den_ps.append(d_ps)
                recip_sb = bld.tile([128, 2, S + 1], fp32, tag="recip")
                for q in range(2):
                    nc.vector.reciprocal_approx_fast(
                        recip_sb[:, q, :], den_ps[q])

                ctx_ps = []
                for q in range(2):
                    c_ps = ps.tile([128, S + 1], fp32, tag="ctx")
                    for cg in range(4):
                        h = 4 * q + cg
                        for ct in range(2):
                            nc.tensor.matmul(
                                c_ps[32 * cg:32 * (cg + 1), :],
                                lhsT=v_sb[:, ct, 32 * h:32 * (h + 1)],
                                rhs=exp_c[ct][:, slot[h], :],
                                start=(ct == 0), stop=(ct == 1),
                                tile_position=(0, 32 * cg))
                    ctx_ps.append(c_ps)

                # ---- blend (normalize) + vbar columns ----
                blend_sb = []
                vbar_sb = []
                for q in range(2):
                    bt = bld.tile([128, S], bf, tag="blend")
                    nc.vector.tensor_mul(bt, ctx_ps[q][:, 0:S],
                                         recip_sb[:, q, 0:S])
                    blend_sb.append(bt)
                    vb = small.tile([128, 1], bf, tag="vbar")
                    nc.vector.tensor_mul(vb, ctx_ps[q][:, S:S + 1],
                                         recip_sb[:, q, S:S + 1])
                    vbar_sb.append(vb)

                if stage < 5:
                    dump(v_sb)
                    continue
                # ---- final projection (pos branch folded in via
                #      stride-0-broadcast vbar weights; bias via ones row) ----
                f_ps = ps.tile([128, 2, D], fp32, tag="final")
                for sc in range(2):
                    nc.tensor.matmul(f_ps[:, sc, :], lhsT=ones_sb[0:1, :],
                                     rhs=outb_sb, start=True, stop=False)
                    for q in range(2):
                        vb = vbar_sb[q]
                        vb_bcast = bass.AP(
                            tensor=vb.tensor, offset=vb.offset,
                            ap=list(vb.ap[:1]) + [[0, 128]])
                        nc.tensor.matmul(
                            f_ps[:, sc, :], lhsT=vb_bcast,
                            rhs=owtg_sb[:, q, :], start=False, stop=False)
                        nc.tensor.matmul(
                            f_ps[:, sc, :],
                            lhsT=blend_sb[q][:, 128 * sc:128 * (sc + 1)],
                            rhs=owt_sb[:, q, :], start=False, stop=(q == 1))
                o_sb = osb.tile([128, 2, D], fp32, tag="o")
                nc.vector.tensor_copy(o_sb, f_ps)
                nc.sync.dma_start(
                    out=out_d[b].rearrange("(c p) d -> p c d", p=128), in_=o_sb)

    nc.finalize()
    return nc


def _prep_inputs(inputs):
    g = 1.0 / (1.0 + np.exp(-inputs["gate"].astype(np.float64)))
    g = g.astype(np.float32)  # [H]
    omg_j = np.repeat(1.0 - g, HD)  # per j = 32h+d'
    gr_j = np.repeat(g / (1.0 - g), HD)

    wqT = inputs["Wq"].T.astype(bf16)
    wkT = inputs["Wk"].T.astype(bf16)
    vT = (inputs["v_embed"].reshape(D, D).T * omg_j[None, :]).astype(bf16)
    owT = inputs["out_w"].T.astype(bf16)
    owTg = (inputs["out_w"].T * gr_j[:, None]).astype(bf16)
    w1T = inputs["pos_w1"].T.astype(bf16)
    b1c = inputs["pos_b1"].reshape(PD, 1).astype(np.float32)
    w2T = inputs["pos_w2"].T.astype(bf16)
    hwN = (-inputs["head_w"].T).astype(bf16)
    hb2c = (-(inputs["head_w"] @ inputs["pos_b2"])).reshape(H, 1).astype(np.float32)
    outbR = inputs["out_b"].reshape(1, D).astype(bf16)
    ident = np.eye(128, dtype=np.float32).astype(bf16)
    ident32 = np.eye(128, dtype=np.float32)
    onesL = np.ones((128, 128), dtype=np.float32).astype(bf16)

    shared = dict(wqT=wqT, wkT=wkT, vT=vT, owT=owT, owTg=owTg, w1T=w1T,
                  b1c=b1c, w2T=w2T, hwNeg=hwN, hb2c=hb2c, outbRow=outbR,
                  ident=ident, ident32=ident32, onesLhs=onesL)

    x = np.ascontiguousarray(inputs["x"], dtype=np.float32)
    pos = np.ascontiguousarray(inputs["pos"], dtype=np.float32)
    in_maps = []
    for c in range(NCORES):
        m = dict(shared)
        m["x"] = np.ascontiguousarray(x[c * NB:(c + 1) * NB])
        m["pos"] = np.ascontiguousarray(pos[c * NB:(c + 1) * NB])
        in_maps.append(m)
    return in_maps


def kernel(**inputs):
    from concourse.bass_utils import run_bass_kernel_spmd

    inputs = {k: np.asarray(v) for k, v in inputs.items()}
    if "nc" not in _CACHE:
        _CACHE["nc"] = _build(NB)
    in_maps = _prep_inputs(inputs)
    res = run_bass_kernel_spmd(_CACHE["nc"], in_maps, core_ids=list(range(NCORES)))
    out = np.concatenate([r["out"] for r in res.results], axis=0)
    return out.astype(np.float32)

